# revision 21
# baseline (speedup 1.0000x reference)
"""Causal self-attention (B=2, T=2048, C=1024, NH=16, HD=64) on 8 trn2 NeuronCores.

Sharding: 2 batch groups x 4 head-groups. Core g (0..7) handles batch b=g//4
and heads [4*(g%4), 4*(g%4)+4). Each core computes its 4 heads' attention and a
partial projection (row-split W_proj); the host sums 4 partials per batch.

v2 pipeline (PE-minimizing; matmul cost ~ N output cols, independent of K):
  - QKV + RoPE as in v1 (even/odd split form, d-on-partitions), but the
    rope'd Q^T/K^T are then REPACKED via sbuf->sbuf DMA into per-head
    K=64 merged tiles ([2 heads x 64 dims, T], dims interleaved e,o =
    original order), so each score matmul is a single K=64 mm instead of
    two K=32 accumulating mms (halves score PE time).
  - Softmax denominators come free from the AV matmul: V carries an extra
    ones-column (M=65); psum row 64 accumulates sum_j exp. No ones-matmul
    per j-chunk (saves ~29us PE).
  - Normalization: denominator row copied to sbuf (ACT Copy, no act-table
    switch), broadcast across 64 partitions with K=1 ones-matmuls, then
    DVE tensor_tensor(divide). Odd heads divide into a temp and DMA-move
    to yab[64:128] (engine lanes are partition-fixed).
  - Scores emitted one j-chunk ahead of AV (software pipeline); next
    time-block's QKV matmuls are emitted before this block's normalize so
    the PE never waits on the ACT/DVE softmax tail.
  - proj: psum -> sbuf bf16 copy (ACT) -> DRAM; host sums bf16 partials.
"""

import numpy as np

B, T, C, NH, HD = 2, 2048, 1024, 16, 64
NCORES = 8
HPC = 4          # heads per core
IBS = 512        # i-block (query block) size
NIB = T // IBS   # 4 i-blocks
JCS = 128        # j-chunk (key chunk) size

_cache = {}


def _patch_tile_drain():
    """This walrus build can't encode multi-wait InstDrain: split the Tile
    tail drain into a chain of single-wait drains."""
    import concourse.tile as _tile
    if getattr(_tile.TileContext, "_drain_patched", False):
        return
    import bass_rust as _br
    from concourse.vector_clock import ScopedClock

    def _drain_and_barrier(self, tick_clock, wait_clock):
        nc = self.nc
        drain_inst = nc.sync.drain()
        wait_clock.add_sem_waits(
            drain_inst.ins, ScopedClock({None: tick_clock.global_clock})
        )
        si = drain_inst.ins.sync_info
        waits = list(si.on_wait or [])
        if len(waits) > 1:
            si.on_wait = waits[:1]
            for w in waits[1:]:
                extra = nc.sync.drain()
                extra.ins.sync_info = _br.SyncInfo(on_wait=[w], on_update=[])
        nc.all_engine_barrier()
        assert self.sems is not None
        popped = nc._tile_sem_poison_stack.pop()
        assert popped is self._sem_poison
        nc.clear_and_free_semaphores(list(self.sems.allocated().values()))
        nc.all_engine_barrier()

    _tile.TileContext._drain_and_barrier = _drain_and_barrier

    # This walrus also refuses >1 sem wait on ANY instruction: peel extra
    # waits onto ENGINE_NOP carriers inserted just before, same engine/bb.
    _orig_add = _tile.TileContext._add_instruction

    def _add_instruction(self, inst):
        si = getattr(inst, "sync_info", None)
        if si is not None and si.on_wait and len(si.on_wait) > 1:
            waits = list(si.on_wait)
            si.on_wait = waits[-1:]
            import concourse.mybir as _mb
            for w in waits[:-1]:
                nop = _mb.InstEventSemaphore(
                    name=self.nc.get_next_instruction_name(), ins=[], outs=[])
                nop.engine = inst.engine
                nop.sync_info = _br.SyncInfo(on_wait=[w], on_update=[])
                _orig_add(self, nop)
        _orig_add(self, inst)

    _tile.TileContext._add_instruction = _add_instruction
    _tile.TileContext._drain_patched = True


def build_nc():
    import concourse.bass as bass
    import concourse.mybir as mybir
    import concourse.tile as tile
    from contextlib import ExitStack

    _patch_tile_drain()
    dt = mybir.dt
    f32, bf16 = dt.float32, dt.bfloat16
    AL = mybir.AluOpType
    Exp = mybir.ActivationFunctionType.Exp
    Ln = mybir.ActivationFunctionType.Ln
    nc = bass.Bass()

    xt = nc.declare_dram_parameter("xt", [C, T], bf16, isOutput=False)
    wd = {}
    for name, w in (("qe", 128), ("qo", 128), ("ke", 128), ("ko", 128),
                    ("v", 256)):
        wd[name] = nc.declare_dram_parameter(f"w{name}", [C, w], bf16,
                                             isOutput=False)
    cosd = nc.declare_dram_parameter("cosd", [128, T], bf16, isOutput=False)
    sind = nc.declare_dram_parameter("sind", [128, T], bf16, isOutput=False)
    maskA = nc.declare_dram_parameter("maskA", [128, 128], bf16, isOutput=False)
    wp = nc.declare_dram_parameter("wp", [256, C], bf16, isOutput=False)
    out = nc.declare_dram_parameter("out", [T, C], bf16, isOutput=True)

    scale = 1.0 / float(np.sqrt(HD))

    with tile.TileContext(nc) as tc, ExitStack() as ctx:
        const = ctx.enter_context(tc.tile_pool(name="const", bufs=1))
        psp = ctx.enter_context(tc.tile_pool(name="psp", bufs=2, space="PSUM"))
        ptp = ctx.enter_context(tc.tile_pool(name="ptp", bufs=4))
        rope_t = ctx.enter_context(tc.tile_pool(name="rope_t", bufs=2))
        dnbp = ctx.enter_context(tc.tile_pool(name="dnbp", bufs=2))
        ynp = ctx.enter_context(tc.tile_pool(name="ynp", bufs=2))
        rbcp = ctx.enter_context(tc.tile_pool(name="rbcp", bufs=2))
        pjp = ctx.enter_context(tc.tile_pool(name="pjp", bufs=4))

        # ---------- constant loads ----------
        # Input DMA order matters: startup is HBM-BW-bound (~19us for 6.8MB).
        # xt chunks are issued on the ACT hwdge queue in parallel with the
        # weight/table loads on the SP queue; q/k weights go first so the
        # chunk-paced QKV matmuls can start as soon as xt chunk 0 lands.
        w_sb = {}
        for name, w in (("ke", 128), ("ko", 128), ("qe", 128), ("qo", 128)):
            t = const.tile([128, 8, w], bf16, tag=f"w_{name}", name=f"w_{name}")
            nc.sync.dma_start(t[:], wd[name][:].rearrange(
                "(cc p) j -> p cc j", p=128))
            w_sb[name] = t
        # xt in four 1MB per-tb slices (qkv(tb) only reads xt[:, tsl]):
        # the first slice + q/k weights (~2.3MB) gate the first matmul
        # instead of the whole 6.8MB input set.
        xt_sb = const.tile([128, 8, T], bf16)
        xt_r = xt[:].rearrange("(cc p) t -> p cc t", p=128)
        nc.scalar.dma_start(xt_sb[:, :, 0:512], xt_r[:, :, 0:512])
        cos_sb = const.tile([128, T], bf16)
        nc.sync.dma_start(cos_sb[:], cosd[:])
        sin_sb = const.tile([128, T], bf16)
        nc.sync.dma_start(sin_sb[:], sind[:])
        nc.scalar.dma_start(xt_sb[:, :, 512:1024], xt_r[:, :, 512:1024])
        wv_sb = const.tile([128, 8, 256], bf16)
        nc.sync.dma_start(wv_sb[:], wd["v"][:].rearrange(
            "(cc p) j -> p cc j", p=128))
        w_sb["v"] = wv_sb
        m0_sb = const.tile([128, 128], bf16)
        nc.sync.dma_start(m0_sb[:], maskA[:])
        nc.scalar.dma_start(xt_sb[:, :, 1024:1536], xt_r[:, :, 1024:1536])
        wp_sb = const.tile([128, 2, C], bf16)
        nc.sync.dma_start(wp_sb[:], wp[:].rearrange("(cc p) co -> p cc co", p=128))
        nc.scalar.dma_start(xt_sb[:, :, 1536:2048], xt_r[:, :, 1536:2048])
        ones_sb = const.tile([128, 64], bf16)
        nc.gpsimd.memset(ones_sb[:], 1.0)

        # rope'd Q^T/K^T in e/o-split layout ([4 heads x 32 freq], {e,o}, T)
        keo = const.tile([128, 2, T], bf16)
        qeo = const.tile([128, 2, T], bf16)
        # merged per-head K=64 layouts: [2 heads x 64 dims, T]
        k_m = [const.tile([128, T], bf16, tag=f"k_m{i}", name=f"k_m{i}")
               for i in range(2)]
        q_m = [const.tile([128, T], bf16, tag=f"q_m{i}", name=f"q_m{i}")
               for i in range(2)]
        # V natural [t, d] with a ones column per head (d index 64)
        v65 = const.tile([128, 16, 4, 65], bf16)
        nc.gpsimd.memset(v65[:, :, :, 64:65], 1.0)
        yab = const.tile([128, T], bf16)          # y^T heads 0,1 (normalized)
        ycd = const.tile([128, T], bf16)          # y^T heads 2,3

        def qkv(tb):
            tsl = slice(tb * 512, (tb + 1) * 512)
            for mk in ("k", "q"):
                ps = psp.tile([128, 2, 512], f32, tag="tr", name="ps_qk")
                for cc in range(8):
                    nc.tensor.matmul(
                        ps[:, 0, :], lhsT=w_sb[mk + "e"][:, cc, :],
                        rhs=xt_sb[:, cc, tsl],
                        start=(cc == 0), stop=(cc == 7))
                for cc in range(8):
                    nc.tensor.matmul(
                        ps[:, 1, :], lhsT=w_sb[mk + "o"][:, cc, :],
                        rhs=xt_sb[:, cc, tsl],
                        start=(cc == 0), stop=(cc == 7))
                eo = keo if mk == "k" else qeo
                # E' = E*cos - O*sin ; O' = O*cos + E*sin
                a = rope_t.tile([128, 512], f32, tag="ra", name="ra")
                nc.vector.tensor_tensor(a[:], ps[:, 0, :], cos_sb[:, tsl],
                                        AL.mult)
                b = rope_t.tile([128, 512], f32, tag="rb", name="rb")
                nc.vector.tensor_tensor(b[:], ps[:, 1, :], sin_sb[:, tsl],
                                        AL.mult)
                nc.vector.tensor_tensor(eo[:, 0, tsl], a[:], b[:], AL.subtract)
                c = rope_t.tile([128, 512], f32, tag="rc", name="rc")
                nc.vector.tensor_tensor(c[:], ps[:, 1, :], cos_sb[:, tsl],
                                        AL.mult)
                d = rope_t.tile([128, 512], f32, tag="rd", name="rd")
                nc.vector.tensor_tensor(d[:], ps[:, 0, :], sin_sb[:, tsl],
                                        AL.mult)
                nc.vector.tensor_tensor(eo[:, 1, tsl], c[:], d[:], AL.add)
                # repack into merged per-head K=64 layout:
                # dst row 64h+2j+s <- eo[32h+j, s, :]  (partition-major stream)
                ms = k_m if mk == "k" else q_m
                nc.sync.dma_start(ms[0][:, tsl], eo[0:64, :, tsl])
                nc.sync.dma_start(ms[1][:, tsl], eo[64:128, :, tsl])
            for half in range(2):
                vt = psp.tile([128, 2, 512], f32, tag="tr", name="vt")
                for ch in range(2):
                    tcx = 4 * tb + 2 * half + ch
                    tchunk = slice(tcx * 128, (tcx + 1) * 128)
                    for cc in range(8):
                        nc.tensor.matmul(
                            vt[:, ch, 0:256], lhsT=xt_sb[:, cc, tchunk],
                            rhs=w_sb["v"][:, cc, :],
                            start=(cc == 0), stop=(cc == 7))
                    nc.vector.tensor_copy(
                        v65[:, tcx, :, 0:64],
                        vt[:, ch, 0:256].rearrange("p (h d) -> p h d", h=4))

        def attn(ib, carry=()):
            """Emit attention for i-block ib. `carry` is a list of deferred
            emitters (previous block's normalize-PE-part and proj chunks);
            one is emitted after every odd jc so slow cross-engine chains
            never sit at the head of the PE queue ahead of ready scores."""
            carry = list(carry)
            isl = slice(ib * IBS, (ib + 1) * IBS)
            y1 = psp.tile([128, 2, 512], f32, tag="y", name="y1")
            y2 = psp.tile([128, 2, 512], f32, tag="y", name="y2")
            njc = 4 * ib + 4
            pend = []

            def emit_av(jc, pt, osl):
                for h in range(4):
                    yt = y1 if h < 2 else y2
                    nc.tensor.matmul(
                        yt[0:65, h % 2, osl],
                        lhsT=v65[:, jc, h, :],
                        rhs=pt[:, h, osl],
                        start=(jc == 0), stop=(jc == njc - 1),
                        skip_group_check=True)

            for jc in range(njc):
                jsl = slice(jc * JCS, (jc + 1) * JCS)
                jcd = jc - 4 * ib
                off = 128 * jcd if jcd > 0 else 0   # fully-masked cols
                osl = slice(off, 512)
                iosl = slice(ib * IBS + off, (ib + 1) * IBS)
                pt = ptp.tile([128, 4, 512], bf16, tag="pt", name="pt")
                for pair, (km, qm) in enumerate(((k_m[0], q_m[0]),
                                                 (k_m[1], q_m[1]))):
                    sc = psp.tile([128, 2, 512], f32, tag="tr", name="sc")
                    for hh in range(2):
                        hsl = slice(64 * hh, 64 * hh + 64)
                        nc.tensor.matmul(
                            sc[:, hh, osl],
                            lhsT=km[hsl, jsl],
                            rhs=qm[hsl, iosl],
                            start=True, stop=True,
                            tile_position=(64 * hh, 0))
                    nc.scalar.activation(pt[:, 2 * pair:2 * pair + 2, osl],
                                         sc[:, :, osl], Exp, scale=scale)
                if jcd >= 0:
                    tsl2 = slice(off, off + 128)
                    nc.vector.tensor_tensor(
                        pt[:, :, tsl2], pt[:, :, tsl2],
                        m0_sb[:, None, :].broadcast_to([128, 4, 128]),
                        AL.mult)
                # consume deferred emitters on late odd jcs so the deferred
                # PE work pads the block-end boundary (where ACT lags most)
                if jc % 2 == 1 and carry and jc >= njc - 2 * len(carry):
                    carry.pop(0)()
                pend.append((jc, pt, osl))
                if len(pend) > 2:
                    emit_av(*pend.pop(0))
            while carry:
                carry.pop(0)()
            for a in pend:
                emit_av(*a)
            # normalize part 1 (ACT only): 1/dn = Exp(-Ln(dn)); runs on the
            # ACT queue while the PE crunches the next block's QKV. Both
            # funcs share the Exp act table -> no table reloads.
            dnbs = []
            for pair, yt in enumerate((y1, y2)):
                lnt = dnbp.tile([128, 2, 512], f32, tag="lnt", name="lnt")
                nc.scalar.activation(lnt[64:65, :, :], yt[64:65, :, :], Ln)
                dnb = dnbp.tile([128, 2, 512], bf16, tag="dnb", name="dnb")
                nc.scalar.activation(dnb[64:65, :, :], lnt[64:65, :, :], Exp,
                                     scale=-1.0)
                dnbs.append(dnb)
            return y1, y2, isl, dnbs

        def normalize2(y1, y2, isl, dnbs):
            """PE/DVE part of normalization: broadcast 1/dn across 64
            partitions with K=1 ones-matmuls, multiply, DMA-move odd heads."""
            for pair, (yt, dnb, ytile) in enumerate(((y1, dnbs[0], yab),
                                                     (y2, dnbs[1], ycd))):
                rb = psp.tile([128, 2, 512], f32, tag="tr", name="rbt")
                for k in range(2):
                    nc.tensor.matmul(
                        rb[0:64, k, :],
                        lhsT=ones_sb[64:65, :],
                        rhs=dnb[64:65, k, :],
                        start=True, stop=True,
                        tile_position=(64, 0),
                        skip_group_check=True)
                rbc = rbcp.tile([64, 2, 512], f32, tag="rbc", name="rbc")
                nc.vector.tensor_copy(rbc[:], rb[0:64, :, :])
                nc.vector.tensor_tensor(ytile[0:64, isl], yt[0:64, 0, :],
                                        rbc[:, 0, :], AL.mult)
                yn = ynp.tile([64, 512], bf16, tag="yn", name="yn")
                nc.vector.tensor_tensor(yn[:], yt[0:64, 1, :],
                                        rbc[:, 1, :], AL.mult)
                nc.sync.dma_start(ytile[64:128, isl], yn[:])

        def proj1(tb, tc4):
            tcx = 4 * tb + tc4
            tchunk = slice(tcx * 128, (tcx + 1) * 128)
            pj = psp.tile([128, 2, 512], f32, tag="tr", name="pj")
            for cob in range(2):
                cosl = slice(cob * 512, (cob + 1) * 512)
                nc.tensor.matmul(
                    pj[:, cob, :], lhsT=yab[:, tchunk],
                    rhs=wp_sb[:, 0, cosl],
                    start=True, stop=False)
                nc.tensor.matmul(
                    pj[:, cob, :], lhsT=ycd[:, tchunk],
                    rhs=wp_sb[:, 1, cosl],
                    start=False, stop=True)
                po = pjp.tile([128, 512], bf16, tag="po", name="po")
                nc.vector.tensor_copy(po[:], pj[:, cob, :])
                nc.sync.dma_start(out[tchunk, cosl], po[:])

        def projs(tb):
            return [lambda t=tb, c=c: proj1(t, c) for c in range(4)]

        # attn(ib) needs qkv(0..ib); process ib0 (shortest, njc=4) LAST so
        # the end-of-kernel exp/normalize tail is as small as possible.
        # normalize2(tb) sits at the qkv->attn boundary (its rb-matmuls
        # cover the rope->repack->scores latency); proj chunks are carried
        # into the next attention block's jc loop.
        qkv(0)
        qkv(1)
        args = attn(1)
        qkv(2)
        normalize2(*args)
        args2 = attn(2, projs(1))
        qkv(3)
        normalize2(*args2)
        args3 = attn(3, projs(2))
        normalize2(*args3)
        args0 = attn(0, projs(3))
        normalize2(*args0)
        for c in range(4):
            proj1(0, c)
    return nc


def _host_prep(x, cos, sin, W_attn, W_proj):
    """Build the 8 per-core input maps (pure data movement / layout prep)."""
    import ml_dtypes
    bf16 = ml_dtypes.bfloat16
    x = np.asarray(x)
    cos = np.asarray(cos)
    sin = np.asarray(sin)
    W_attn = np.asarray(W_attn)
    W_proj = np.asarray(W_proj)

    cosf = np.ascontiguousarray(cos[0, 0][:, 0::2].T)  # [32,T]
    sinf = np.ascontiguousarray(sin[0, 0][:, 0::2].T)
    cosd = np.tile(cosf, (4, 1)).astype(bf16)  # [128, T]
    sind = np.tile(sinf, (4, 1)).astype(bf16)

    mA = (np.arange(128)[:, None] <= np.arange(128)[None, :]).astype(np.float32)

    ev = np.arange(0, HD, 2)
    od = np.arange(1, HD, 2)
    Wq, Wk, Wv = W_attn[:, 0:C], W_attn[:, C:2 * C], W_attn[:, 2 * C:3 * C]
    xt = [np.ascontiguousarray(x[b].T).astype(bf16) for b in range(B)]

    in_maps = []
    for g in range(NCORES):
        b, hg = g // HPC, g % HPC
        heads = [HPC * hg + i for i in range(HPC)]
        mk = lambda W, idx: np.ascontiguousarray(
            np.concatenate([W[:, 64 * h + idx] for h in heads], 1)).astype(bf16)
        in_maps.append({
            "xt": xt[b],
            "wqe": mk(Wq, ev), "wqo": mk(Wq, od),
            "wke": mk(Wk, ev), "wko": mk(Wk, od),
            "wv": mk(Wv, np.arange(HD)),
            "cosd": cosd, "sind": sind, "maskA": mA.astype(bf16),
            "wp": np.ascontiguousarray(np.concatenate(
                [W_proj[64 * h:64 * h + 64, :] for h in heads], 0)
            ).astype(bf16),
        })
    return in_maps


def _run(inputs, trace=False):
    from concourse.bass_utils import run_bass_kernel_spmd

    if "nc" not in _cache:
        _cache["nc"] = build_nc()
    nc = _cache["nc"]
    in_maps = _host_prep(**inputs)
    res = run_bass_kernel_spmd(
        nc, in_maps, core_ids=list(range(NCORES)), trace=trace)
    outp = np.stack([np.asarray(res.results[g]["out"], dtype=np.float32)
                     for g in range(NCORES)])
    full = np.stack([outp[4 * b:4 * b + 4].sum(axis=0) for b in range(B)])
    return full, res


def kernel(**inputs):
    full, _ = _run(inputs, trace=False)
    return full
